# revision 39
# baseline (speedup 1.0000x reference)
"""Trainium2 Bass kernel for nn_Attention2 (dense transformer block with
softmax over the heads axis).

Computation per (n, t) batch b (B = n*t = 4096 total, X_b = x[n,:,t,:].T is
[vv=25, c=512]):
    qkv = X_b @ w_qkv.T, split into q,k,v heads [h=8, 25, hd=64]
    s[h,i,j] = (q[h,i,:] . k[h,j,:]) / 8      (scale folded into w_q on host)
    p = softmax over h (axis 0)
    o[h,i,:] = sum_j p[h,i,j] v[h,j,:]  -> [25, 512] -> @ w_proj.T
    out[n,:,t,:] = result.T

Sharding: data-parallel over n, 2 n-values (512 batches) per core, 8 cores.

Pipelining (898us group-serial baseline -> 552us): the PE must never idle
>~3.4us or the HAM clock gate re-throttles it to 1.2GHz, and the per-sub
softmax chain (exp->reduce->recip->mul) takes ~2.5-3us including the
~26ns-per-instruction semaphore-post tails after each 32-matmul wave.  So
the kernel runs two pipeline levels deep:
  - group level: group g+1's qkv GEMMs are emitted interleaved into group
    g's attention core as PE filler (emission order = Tile scheduler
    priority);
  - sub level: attnv(g,s) is emitted one sub after its scores(g,s), and
    attnv(g,3) + proj(g) land inside group g+1's sub 0, so every softmax
    chain hides under a full sub (~3us) of filler GEMMs and there is no
    group-junction stall.
Other key choices: scores waves are par-major (exp(par0) waits only half
the sem-post tail); attnv packs 4 row-groups x 2 col-groups of the PE
array (8-way concurrent quadrant matmuls) with one PSUM bank per
row-group; proj's pf tiles ride the po banks (the bank WAR coincides with
proj's real oT dependency); PSUM budget is pbig 2 + psm 2 + po/pf 4 = 8
banks; x is re-laid-out host-side to [nn,c,group,TG,32] so every x DMA is
a contiguous 1KiB-per-partition transfer; all GEMM operands are fp16
(1 cycle/row); output is stored fp16 and cast to fp32 on host; dummy
matmuls keep the PE warm during the prologue weight DMAs and the final
group's chains.
"""
import numpy as np
import concourse.bass as bass
import concourse.mybir as mybir
import concourse.tile as tile
from concourse.bass_utils import run_bass_kernel_spmd
from concourse.vector_clock import ScopedClock, VectorClock

F32 = mybir.dt.float32
F16 = mybir.dt.float16

N_CORES = 8
NN_PER_CORE = 2        # n values per core
T = 256
VV = 25
C = 512
H = 8
HD = 64
TG = 16                # t values (batches) per group
NGROUPS = NN_PER_CORE * (T // TG)   # 32 groups per core
NB = TG * VV           # 400 moving columns per group
NSUB = 4               # sub-blocks of 4 batches per group


def _split_drain_and_barrier(self, tick_clock, wait_clock):
    # walrus caps sync-wait commands at 1 for CTRL_NO; split the kernel-tail
    # drain into one drain per pending proc.
    vc = tick_clock.global_clock
    n = len(vc)
    for i in range(n):
        if vc[i] == 0:
            continue
        sub = VectorClock([vc[j] if j == i else 0 for j in range(n)])
        d = self.nc.sync.drain()
        wait_clock.add_sem_waits(d.ins, ScopedClock({None: sub}))
    self.nc.all_engine_barrier()
    assert self.sems is not None
    popped = self.nc._tile_sem_poison_stack.pop()
    assert popped is self._sem_poison
    self.nc.clear_and_free_semaphores(list(self.sems.allocated().values()))
    self.nc.all_engine_barrier()


tile.TileContext._drain_and_barrier = _split_drain_and_barrier


def split_excess_waits(nc, limit=1):
    """walrus codegen allows very few sync-wait commands per instruction
    (1 for matmul/drain/DMA structs).  Move excess waits onto same-engine
    NoOp carriers inserted just before the instruction — same semantics,
    since each engine executes its queue in order."""
    k = 0
    for fn in nc.m.functions:
        for bb in fn.blocks:
            out = []
            for ins in bb.instructions:
                si = ins.sync_info
                waits = list(si.on_wait) if si is not None and si.on_wait else []
                if len(waits) > limit:
                    keep = waits[-limit:]
                    for w in waits[:-limit]:
                        nop = mybir.InstNoOp(
                            name=f"WC-{k}", ins=[], outs=[], engine=ins.engine
                        )
                        k += 1
                        nop.sync_info = mybir.SyncInfo(on_wait=[w], on_update=[])
                        out.append(nop)
                    si.on_wait = keep
                out.append(ins)
            bb.instructions[:] = out
    return k


def build_nc():
    nc = bass.Bass()
    # x is re-laid-out on host to [nn, c, group, TG, 32] (32 = VV zero-padded)
    # so each x-slab DMA is a contiguous 1KiB-per-partition transfer; the
    # padded tail doubles as clean zeros for the v-matmul stationary.
    X = nc.declare_dram_parameter(
        "x", [NN_PER_CORE, C, T // TG, TG, 32], F16, isOutput=False)
    WQK = nc.declare_dram_parameter("wqkT", [C, 2 * C], F16, isOutput=False)
    WV = nc.declare_dram_parameter("wvT", [C, C], F16, isOutput=False)
    WP = nc.declare_dram_parameter("wprojT", [C, C], F16, isOutput=False)
    Y = nc.declare_dram_parameter("y", [NN_PER_CORE, C, T, VV], F16, isOutput=True)

    with tile.TileContext(nc) as tc:
        with (
            tc.tile_pool(name="consts", bufs=1) as consts,
            tc.tile_pool(name="xpool", bufs=3) as xpool,
            tc.tile_pool(name="qpool", bufs=2) as qpool,
            tc.tile_pool(name="vpool", bufs=2) as vpool,
            tc.tile_pool(name="smpool", bufs=2) as smpool,
            tc.tile_pool(name="opool", bufs=2) as opool,
            tc.tile_pool(name="fpool", bufs=2) as fpool,
            tc.tile_pool(name="pbig", bufs=2, space="PSUM") as pbig,
            tc.tile_pool(name="psmall", bufs=1, space="PSUM") as psmall,
        ):
            # ---- weight loads (wqk first: the first qk chunk needs all 4) ----
            wqk_r, wv_r, wp_r = [], [], []
            for kc in range(4):
                r0 = consts.tile([128, 2 * C], F16, tag=f"wqkr{kc}")
                nc.sync.dma_start(out=r0, in_=WQK[kc * 128:(kc + 1) * 128, :])
                wqk_r.append(r0)

            # ---- PE warmup: ~10us of dummy matmuls overlapping the weight
            # and x(0) DMAs so the HAM clock gate reaches 8/8 before group 0 ----
            wu = consts.tile([128, C], F16, tag="warm")
            nc.vector.memset(wu[:], 0.0)
            for _ in range(14):
                pwu = pbig.tile([128, C], F32, tag="big", name="pwu")
                nc.tensor.matmul(pwu[:], wu[:, 0:128], wu[:],
                                 start=True, stop=True)

            def load_x(g, eng=None):
                eng = eng or nc.sync
                xq = []
                nn = g // (T // TG)
                gi = g % (T // TG)
                for kc in range(4):
                    xt = xpool.tile([128, TG, 32], F16, tag=f"xp{kc}", name="xt")
                    eng.dma_start(
                        out=xt[:],
                        in_=X[nn, kc * 128:(kc + 1) * 128, gi, :, :],
                    )
                    xq.append(xt)
                return xq

            def emit_qk_chunk(m, xp, qc_next):
                # q^T/k^T chunk m: c'-rows m*128..m*128+128, cols = (b, i)
                pq = pbig.tile([128, NB], F32, tag="big", name="pq")
                for kc in range(4):
                    nc.tensor.matmul(
                        pq[:],
                        wqk_r[kc][:, m * 128:(m + 1) * 128],
                        xp[kc][:, :, 0:VV],
                        start=(kc == 0), stop=(kc == 3),
                    )
                # evac engine alternates by m parity so every sub's two qk
                # chunks split 1 ACT + 1 DVE; score MMs for head-pair hp read
                # qc[hp] and qc[hp+4] (same parity -> same producing engine,
                # keeping each matmul at a single cross-engine wait)
                qcm = qpool.tile([128, NB], F16, tag=f"qkT{m}", name="qcm")
                if m % 2 == 0:
                    nc.scalar.activation(
                        qcm[:], pq[:], mybir.ActivationFunctionType.Copy
                    )
                else:
                    nc.vector.tensor_copy(qcm[:], pq[:])
                qc_next[m] = qcm

            def emit_v_sub(s, xp, v2_next):
                # v for batches s*4..s*4+4 in [token, c'] layout; token row
                # b4*32+j so attnv lhsT slices sit at 32-aligned bases that
                # match their tile_position row
                pv = pbig.tile([128, C], F32, tag="big", name="pv")
                for kc in range(4):
                    nc.tensor.matmul(
                        pv[:],
                        xp[kc][:, s * 4:s * 4 + 4, :],
                        wv_r[kc][:],
                        start=(kc == 0), stop=(kc == 3),
                    )
                v2 = vpool.tile([128, C], F16, tag=f"v2_{s}", name="v2")
                nc.scalar.activation(
                    v2[:], pv[:], mybir.ActivationFunctionType.Copy,
                )
                v2_next[s] = v2

            def emit_dummy_qk(xp):
                # last-group PE filler: keeps the HAM clock gate warm through
                # the final group's softmax chains; result never read
                pq = pbig.tile([128, NB], F32, tag="big", name="pqd")
                for kc in range(4):
                    nc.tensor.matmul(
                        pq[:],
                        wqk_r[kc][:, 0:128],
                        xp[kc][:, :, 0:VV],
                        start=(kc == 0), stop=(kc == 3),
                    )

            def emit_dummy_v(xp):
                pv = pbig.tile([128, C], F32, tag="big", name="pvd")
                for kc in range(4):
                    nc.tensor.matmul(
                        pv[:],
                        xp[kc][:, 0:4, :],
                        wv_r[kc][:],
                        start=(kc == 0), stop=(kc == 3),
                    )

            def emit_scores(s, qc, psm):
                # par-major: all par0 matmuls first so exp(par0) waits only
                # half the wave's (serialized ~26ns-each) sem increments
                for par in range(2):
                    r0 = par * 64
                    for m in range(4):
                        for b4 in range(4):
                            bcol = (s * 4 + b4) * VV
                            nc.tensor.matmul(
                                psm[par][b4 * 32:b4 * 32 + 25, m, :],
                                qc[4 + m][r0:r0 + 64, bcol:bcol + VV],
                                qc[m][r0:r0 + 64, bcol:bcol + VV],
                                start=True, stop=True,
                                tile_position=(r0, b4 * 32),
                            )

            def emit_attnv(s, v2s, p2, po):
                # 4 row-groups (b4*32) x 2 col-groups -> 8-way concurrent
                # quadrant waves; each row-group drains into its own po bank
                for b4 in range(4):
                    for h in range(H):
                        m, c0 = h // 2, (h % 2) * 64
                        nc.tensor.matmul(
                            po[b4][c0:c0 + 64, m, :],
                            v2s[b4 * 32:b4 * 32 + 25, h * HD:(h + 1) * HD],
                            p2[b4 * 32:b4 * 32 + 25, h % 2, h // 2, :],
                            start=True, stop=True,
                            tile_position=(b4 * 32, c0),
                        )

            # ---- prologue: x(0), remaining weights, qkv(0), x(1) ----
            xp_cur = load_x(0)          # consumed by group 0
            for kc in range(4):
                r1 = consts.tile([128, C], F16, tag=f"wvr{kc}")
                nc.sync.dma_start(out=r1, in_=WV[kc * 128:(kc + 1) * 128, :])
                wv_r.append(r1)
            for kc in range(4):
                r2 = consts.tile([128, C], F16, tag=f"wpr{kc}")
                nc.sync.dma_start(out=r2, in_=WP[kc * 128:(kc + 1) * 128, :])
                wp_r.append(r2)
            qc_cur = [None] * 8
            v2_cur = [None] * NSUB
            for s in range(NSUB):
                emit_qk_chunk(2 * s, xp_cur, qc_cur)
                emit_qk_chunk(2 * s + 1, xp_cur, qc_cur)
                emit_v_sub(s, xp_cur, v2_cur)
            xp_a = load_x(1)            # for qkv(1), emitted in iter 0

            def emit_pending_attnv(pend):
                # attention @ v wave for (g', s') = pend, one sub after its
                # scores so the softmax chain hides under a full sub of
                # filler GEMMs
                sp, p2p, v2p, oTp = pend
                po = [
                    psmall.tile([128, 4, VV], F32, tag=f"att{b4}",
                                name=f"po{b4}",
                                padded_shape=[128, 4, 128])
                    for b4 in range(4)
                ]
                emit_attnv(sp, v2p, p2p, po)
                # evacuate po -> oT (fp16), split DVE/ACT
                oTr = oTp[:].rearrange("p m (b i) -> p m b i", i=VV)
                for b4 in range(4):
                    dst = oTr[:, :, sp * 4 + b4, :]
                    if b4 % 2 == 0:
                        nc.vector.tensor_copy(dst, po[b4][:])
                    else:
                        nc.scalar.activation(
                            dst, po[b4][:],
                            mybir.ActivationFunctionType.Copy,
                        )

            def emit_proj(oTp, nn, t0):
                # pf rides the po banks: its WAR (oT evac of sub3 freeing the
                # bank) coincides with proj's real data dependency on oT
                for co in range(4):
                    pf = psmall.tile([128, NB], F32, tag=f"att{co % 2 * 2 + 1}",
                                     name="pf", padded_shape=[128, 512])
                    for kc in range(4):
                        nc.tensor.matmul(
                            pf[:],
                            wp_r[kc][:, co * 128:(co + 1) * 128],
                            oTp[:, kc, :],
                            start=(kc == 0), stop=(kc == 3),
                        )
                    fin = fpool.tile([128, NB], F16, tag=f"fin{co}", name="fin")
                    if co < 2:
                        nc.vector.tensor_copy(fin[:], pf[:])
                    else:
                        nc.scalar.activation(
                            fin[:], pf[:], mybir.ActivationFunctionType.Copy
                        )
                    nc.sync.dma_start(
                        out=Y[nn, co * 128:(co + 1) * 128, t0:t0 + TG, :],
                        in_=fin[:].rearrange("p (t v) -> p t v", t=TG),
                    )

            # ---- main pipelined loop.  Two pipeline levels: group g+1's
            # qkv GEMMs interleave into group g's attention core as PE
            # filler, and attnv runs one sub behind its scores (attnv(g,3)
            # and proj(g) land in group g+1's sub 0) so every softmax chain
            # (exp->reduce->recip->mul, ~2.5-3us incl. sem-post tails) is
            # covered by ~3us of filler GEMMs with no group-junction stall.
            pending = None          # (s, p2, v2s, oT) awaiting attnv
            proj_todo = None        # (oT, nn, t0) awaiting proj emission
            for g in range(NGROUPS):
                nn = g // (T // TG)
                t0 = (g % (T // TG)) * TG
                have_next = g + 1 < NGROUPS

                if g + 2 < NGROUPS:
                    xp_b = load_x(g + 2)
                else:
                    xp_b = None
                qc_next = [None] * 8
                v2_next = [None] * NSUB
                oT = opool.tile([128, 4, NB], F16, tag="oT", name="oT")

                for s in range(NSUB):
                    # scores wave for (g, s).  PSUM budget: pbig 2 + psm 2 +
                    # po/pf 4 = 8 banks.  psm banks are dedicated so scores
                    # never wait on oT evacuation.
                    psm = [
                        psmall.tile([128, 4, VV], F32, tag=f"psm{par}",
                                    name=f"psm{par}",
                                    padded_shape=[128, 4, 128])
                        for par in range(2)
                    ]
                    emit_scores(s, qc_cur, psm)

                    # softmax over heads (axis split across psm[0]/psm[1])
                    e_t = smpool.tile([128, 2, 4, VV], F16, tag=f"e{s}", name="e_t")
                    for par in range(2):
                        nc.scalar.activation(
                            e_t[:, par], psm[par][:],
                            mybir.ActivationFunctionType.Exp,
                        )

                    # previous sub's attention @ v (pipelined one sub deep)
                    if pending is not None:
                        was_tail = pending[0] == NSUB - 1
                        emit_pending_attnv(pending)
                        if was_tail:
                            assert proj_todo is None
                            proj_todo = (pending[3], nn_prev, t0_prev)
                        pending = None

                    # PE filler: next group's qk chunks, front-loaded
                    # [3,2,2,1] across subs so the last chunk's evacuation
                    # (which gates the next group's scores wave) lands a full
                    # sub earlier and ahead of the softmax ops in the DVE
                    # queue
                    sched = ((0, 1, 2), (3, 4), (5, 6), (7,))[s]
                    if have_next:
                        emit_qk_chunk(sched[0], xp_a, qc_next)
                    else:
                        emit_dummy_qk(xp_cur)

                    # v(g+1, s) here (not after the softmax ops) so its PSUM
                    # slot-WAR resolves a full chunk earlier
                    if have_next:
                        emit_v_sub(s, xp_a, v2_next)
                    else:
                        emit_dummy_v(xp_cur)

                    D = smpool.tile([128, VV], F32, tag=f"D{s}", name="D")
                    nc.vector.reduce_sum(
                        out=D[:],
                        in_=e_t[:].rearrange("p a m i -> p i (a m)"),
                        axis=mybir.AxisListType.X,
                    )
                    rD = smpool.tile([128, VV], F16, tag=f"rD{s}", name="rD")
                    with nc.allow_low_precision(reason="1/D in fp16: D in [2e-2, 3e3], rel err ~5e-4 vs 2e-2 budget"):
                        nc.vector.reciprocal(rD[:], D[:])
                    p2 = smpool.tile([128, 2, 4, VV], F16, tag=f"p2_{s}",
                                     name="p2")
                    nc.vector.tensor_mul(
                        p2[:],
                        e_t[:],
                        rD[:].unsqueeze(1).unsqueeze(1)
                        .broadcast_to([128, 2, 4, VV]),
                    )

                    # PE fillers: next group's remaining qk chunks
                    if have_next:
                        for m in sched[1:]:
                            emit_qk_chunk(m, xp_a, qc_next)
                    else:
                        emit_dummy_qk(xp_cur)

                    # previous group's proj, placed after this sub's fillers
                    # so it never waits on the just-issued oT evacuations
                    if proj_todo is not None:
                        emit_proj(*proj_todo)
                        proj_todo = None

                    pending = (s, p2, v2_cur[s], oT)

                nn_prev, t0_prev = nn, t0
                qc_cur = qc_next
                v2_cur = v2_next
                if xp_a is not None:
                    xp_cur = xp_a
                xp_a = xp_b

            # ---- flush: last group's final attnv + proj ----
            emit_dummy_qk(xp_cur)
            emit_pending_attnv(pending)
            emit_dummy_v(xp_cur)
            emit_proj(pending[3], nn_prev, t0_prev)
    return nc


LAST_RESULT = {}


def kernel(x: np.ndarray, w_qkv: np.ndarray, w_proj: np.ndarray,
           _trace: bool = False) -> np.ndarray:
    n, c, t, vv = x.shape
    assert (n, c, t, vv) == (16, 512, 256, 25)
    scale = np.float32((c // H) ** -0.5)

    wq = w_qkv[:c] * scale
    wk = w_qkv[c:2 * c]
    wv = w_qkv[2 * c:]
    wqkT = np.ascontiguousarray(np.concatenate([wq, wk], axis=0).T.astype(np.float16))
    wvT = np.ascontiguousarray(wv.T.astype(np.float16))
    wprojT = np.ascontiguousarray(w_proj.T.astype(np.float16))

    nc = build_nc()
    split_excess_waits(nc)
    in_maps = []
    for core in range(N_CORES):
        shard = x[core * NN_PER_CORE:(core + 1) * NN_PER_CORE].astype(np.float16)
        xs = np.zeros((NN_PER_CORE, C, T // TG, TG, 32), dtype=np.float16)
        xs[..., :VV] = shard.reshape(NN_PER_CORE, C, T // TG, TG, VV)
        in_maps.append({"x": xs, "wqkT": wqkT, "wvT": wvT, "wprojT": wprojT})

    kw = {}
    if _trace:
        import tempfile
        kw = dict(trace=True, tmpdir=tempfile.mkdtemp(prefix="attn2_trace_"))
    res = run_bass_kernel_spmd(nc, in_maps, list(range(N_CORES)), **kw)
    LAST_RESULT["res"] = res
    LAST_RESULT["tmpdir"] = kw.get("tmpdir")
    out = np.empty((n, c, t, vv), dtype=np.float32)
    for core in range(N_CORES):
        out[core * NN_PER_CORE:(core + 1) * NN_PER_CORE] = \
            res.results[core]["y"].astype(np.float32)
    return out


# revision 40
# speedup vs baseline: 1.1238x; 1.1238x over previous
"""Trainium2 Bass kernel for nn_Attention2 (dense transformer block with
softmax over the heads axis).

Computation per (n, t) batch b (B = n*t = 4096 total, X_b = x[n,:,t,:].T is
[vv=25, c=512]):
    qkv = X_b @ w_qkv.T, split into q,k,v heads [h=8, 25, hd=64]
    s[h,i,j] = (q[h,i,:] . k[h,j,:]) / 8      (scale folded into w_q on host)
    p = softmax over h (axis 0)
    o[h,i,:] = sum_j p[h,i,j] v[h,j,:]  -> [25, 512] -> @ w_proj.T
    out[n,:,t,:] = result.T

Sharding: data-parallel over n, 2 n-values (512 batches) per core, 8 cores.

Pipelining (898us group-serial baseline -> 552us): the PE must never idle
>~3.4us or the HAM clock gate re-throttles it to 1.2GHz, and the per-sub
softmax chain (exp->reduce->recip->mul) takes ~2.5-3us including the
~26ns-per-instruction semaphore-post tails after each 32-matmul wave.  So
the kernel runs two pipeline levels deep:
  - group level: group g+1's qkv GEMMs are emitted interleaved into group
    g's attention core as PE filler (emission order = Tile scheduler
    priority);
  - sub level: attnv(g,s) is emitted one sub after its scores(g,s), and
    attnv(g,3) + proj(g) land inside group g+1's sub 0, so every softmax
    chain hides under a full sub (~3us) of filler GEMMs and there is no
    group-junction stall.
Other key choices: scores waves are par-major (exp(par0) waits only half
the sem-post tail); attnv packs 4 row-groups x 2 col-groups of the PE
array (8-way concurrent quadrant matmuls) with one PSUM bank per
row-group; proj's pf tiles ride the po banks (the bank WAR coincides with
proj's real oT dependency); PSUM budget is pbig 2 + psm 2 + po/pf 4 = 8
banks; x is re-laid-out host-side to [nn,c,group,TG,32] so every x DMA is
a contiguous 1KiB-per-partition transfer; all GEMM operands are fp16
(1 cycle/row); output is stored fp16 and cast to fp32 on host; dummy
matmuls keep the PE warm during the prologue weight DMAs and the final
group's chains.
"""
import numpy as np
import concourse.bass as bass
import concourse.mybir as mybir
import concourse.tile as tile
from concourse.bass_utils import run_bass_kernel_spmd
from concourse.vector_clock import ScopedClock, VectorClock

F32 = mybir.dt.float32
F16 = mybir.dt.float16

N_CORES = 8
NN_PER_CORE = 2        # n values per core
T = 256
VV = 25
C = 512
H = 8
HD = 64
TG = 16                # t values (batches) per group
NGROUPS = NN_PER_CORE * (T // TG)   # 32 groups per core
NB = TG * VV           # 400 moving columns per group
NSUB = 4               # sub-blocks of 4 batches per group


def _split_drain_and_barrier(self, tick_clock, wait_clock):
    # walrus caps sync-wait commands at 1 for CTRL_NO; split the kernel-tail
    # drain into one drain per pending proc.
    vc = tick_clock.global_clock
    n = len(vc)
    for i in range(n):
        if vc[i] == 0:
            continue
        sub = VectorClock([vc[j] if j == i else 0 for j in range(n)])
        d = self.nc.sync.drain()
        wait_clock.add_sem_waits(d.ins, ScopedClock({None: sub}))
    self.nc.all_engine_barrier()
    assert self.sems is not None
    popped = self.nc._tile_sem_poison_stack.pop()
    assert popped is self._sem_poison
    self.nc.clear_and_free_semaphores(list(self.sems.allocated().values()))
    self.nc.all_engine_barrier()


tile.TileContext._drain_and_barrier = _split_drain_and_barrier


def split_excess_waits(nc, limit=1):
    """walrus codegen allows very few sync-wait commands per instruction
    (1 for matmul/drain/DMA structs).  Move excess waits onto same-engine
    NoOp carriers inserted just before the instruction — same semantics,
    since each engine executes its queue in order."""
    k = 0
    for fn in nc.m.functions:
        for bb in fn.blocks:
            out = []
            for ins in bb.instructions:
                si = ins.sync_info
                waits = list(si.on_wait) if si is not None and si.on_wait else []
                if len(waits) > limit:
                    keep = waits[-limit:]
                    for w in waits[:-limit]:
                        nop = mybir.InstNoOp(
                            name=f"WC-{k}", ins=[], outs=[], engine=ins.engine
                        )
                        k += 1
                        nop.sync_info = mybir.SyncInfo(on_wait=[w], on_update=[])
                        out.append(nop)
                    si.on_wait = keep
                out.append(ins)
            bb.instructions[:] = out
    return k


def build_nc():
    nc = bass.Bass()
    # x is re-laid-out on host to [nn, c, group, TG, 32] (32 = VV zero-padded)
    # so each x-slab DMA is a contiguous 1KiB-per-partition transfer; the
    # padded tail doubles as clean zeros for the v-matmul stationary.
    X = nc.declare_dram_parameter(
        "x", [NN_PER_CORE, C, T // TG, TG, 32], F16, isOutput=False)
    WQK = nc.declare_dram_parameter("wqkT", [C, 2 * C], F16, isOutput=False)
    WV = nc.declare_dram_parameter("wvT", [C, C], F16, isOutput=False)
    WP = nc.declare_dram_parameter("wprojT", [C, C], F16, isOutput=False)
    Y = nc.declare_dram_parameter("y", [NN_PER_CORE, C, T, VV], F16, isOutput=True)

    with tile.TileContext(nc) as tc:
        with (
            tc.tile_pool(name="consts", bufs=1) as consts,
            tc.tile_pool(name="xpool", bufs=3) as xpool,
            tc.tile_pool(name="qpool", bufs=2) as qpool,
            tc.tile_pool(name="vpool", bufs=2) as vpool,
            tc.tile_pool(name="smpool", bufs=2) as smpool,
            tc.tile_pool(name="opool", bufs=2) as opool,
            tc.tile_pool(name="fpool", bufs=2) as fpool,
            tc.tile_pool(name="pbig", bufs=2, space="PSUM") as pbig,
            tc.tile_pool(name="psmall", bufs=1, space="PSUM") as psmall,
        ):
            # ---- weight loads (wqk first: the first qk chunk needs all 4) ----
            wqk_r, wv_r, wp_r = [], [], []
            for kc in range(4):
                r0 = consts.tile([128, 2 * C], F16, tag=f"wqkr{kc}")
                nc.sync.dma_start(out=r0, in_=WQK[kc * 128:(kc + 1) * 128, :])
                wqk_r.append(r0)

            # ---- PE warmup: ~10us of dummy matmuls overlapping the weight
            # and x(0) DMAs so the HAM clock gate reaches 8/8 before group 0 ----
            wu = consts.tile([128, C], F16, tag="warm")
            nc.vector.memset(wu[:], 0.0)
            for _ in range(14):
                pwu = pbig.tile([128, C], F32, tag="big", name="pwu")
                nc.tensor.matmul(pwu[:], wu[:, 0:128], wu[:],
                                 start=True, stop=True)

            def load_x(g, eng=None):
                eng = eng or nc.sync
                xq = []
                nn = g // (T // TG)
                gi = g % (T // TG)
                for kc in range(4):
                    xt = xpool.tile([128, TG, 32], F16, tag=f"xp{kc}", name="xt")
                    eng.dma_start(
                        out=xt[:],
                        in_=X[nn, kc * 128:(kc + 1) * 128, gi, :, :],
                    )
                    xq.append(xt)
                return xq

            def emit_qk_chunk(m, xp, qc_next):
                # q^T/k^T chunk m: c'-rows m*128..m*128+128, cols = (b, i)
                pq = pbig.tile([128, NB], F32, tag="big", name="pq")
                for kc in range(4):
                    nc.tensor.matmul(
                        pq[:],
                        wqk_r[kc][:, m * 128:(m + 1) * 128],
                        xp[kc][:, :, 0:VV],
                        start=(kc == 0), stop=(kc == 3),
                    )
                # evac engine alternates by m parity so every sub's two qk
                # chunks split 1 ACT + 1 DVE; score MMs for head-pair hp read
                # qc[hp] and qc[hp+4] (same parity -> same producing engine,
                # keeping each matmul at a single cross-engine wait)
                qcm = qpool.tile([128, NB], F16, tag=f"qkT{m}", name="qcm")
                if m % 2 == 0:
                    nc.scalar.activation(
                        qcm[:], pq[:], mybir.ActivationFunctionType.Copy
                    )
                else:
                    nc.vector.tensor_copy(qcm[:], pq[:])
                qc_next[m] = qcm

            def emit_v_sub(s, xp, v2_next):
                # v for batches s*4..s*4+4 in [token, c'] layout; token row
                # b4*32+j so attnv lhsT slices sit at 32-aligned bases that
                # match their tile_position row
                pv = pbig.tile([128, C], F32, tag="big", name="pv")
                for kc in range(4):
                    nc.tensor.matmul(
                        pv[:],
                        xp[kc][:, s * 4:s * 4 + 4, :],
                        wv_r[kc][:],
                        start=(kc == 0), stop=(kc == 3),
                    )
                v2 = vpool.tile([128, C], F16, tag=f"v2_{s}", name="v2")
                nc.scalar.activation(
                    v2[:], pv[:], mybir.ActivationFunctionType.Copy,
                )
                v2_next[s] = v2

            def emit_dummy_qk(xp):
                # last-group PE filler: keeps the HAM clock gate warm through
                # the final group's softmax chains; result never read
                pq = pbig.tile([128, NB], F32, tag="big", name="pqd")
                for kc in range(4):
                    nc.tensor.matmul(
                        pq[:],
                        wqk_r[kc][:, 0:128],
                        xp[kc][:, :, 0:VV],
                        start=(kc == 0), stop=(kc == 3),
                    )

            def emit_dummy_v(xp):
                pv = pbig.tile([128, C], F32, tag="big", name="pvd")
                for kc in range(4):
                    nc.tensor.matmul(
                        pv[:],
                        xp[kc][:, 0:4, :],
                        wv_r[kc][:],
                        start=(kc == 0), stop=(kc == 3),
                    )

            def emit_scores(s, qc, psm):
                # par-major: all par0 matmuls first so exp(par0) waits only
                # half the wave's (serialized ~26ns-each) sem increments
                for par in range(2):
                    r0 = par * 64
                    for m in range(4):
                        for b4 in range(4):
                            bcol = (s * 4 + b4) * VV
                            nc.tensor.matmul(
                                psm[par][b4 * 32:b4 * 32 + 25, m, :],
                                qc[4 + m][r0:r0 + 64, bcol:bcol + VV],
                                qc[m][r0:r0 + 64, bcol:bcol + VV],
                                start=True, stop=True,
                                tile_position=(r0, b4 * 32),
                            )

            def emit_attnv(s, v2s, p2, po):
                # 4 row-groups (b4*32) x 2 col-groups -> 8-way concurrent
                # quadrant waves; each row-group drains into its own po bank
                for b4 in range(4):
                    for h in range(H):
                        m, c0 = h // 2, (h % 2) * 64
                        nc.tensor.matmul(
                            po[b4][c0:c0 + 64, m, :],
                            v2s[b4 * 32:b4 * 32 + 25, h * HD:(h + 1) * HD],
                            p2[b4 * 32:b4 * 32 + 25, h % 2, h // 2, :],
                            start=True, stop=True,
                            tile_position=(b4 * 32, c0),
                        )

            # ---- prologue: x(0), remaining weights, qkv(0), x(1) ----
            xp_cur = load_x(0)          # consumed by group 0
            for kc in range(4):
                r1 = consts.tile([128, C], F16, tag=f"wvr{kc}")
                nc.sync.dma_start(out=r1, in_=WV[kc * 128:(kc + 1) * 128, :])
                wv_r.append(r1)
            for kc in range(4):
                r2 = consts.tile([128, C], F16, tag=f"wpr{kc}")
                nc.sync.dma_start(out=r2, in_=WP[kc * 128:(kc + 1) * 128, :])
                wp_r.append(r2)
            qc_cur = [None] * 8
            v2_cur = [None] * NSUB
            for s in range(NSUB):
                emit_qk_chunk(2 * s, xp_cur, qc_cur)
                emit_qk_chunk(2 * s + 1, xp_cur, qc_cur)
                emit_v_sub(s, xp_cur, v2_cur)
            xp_a = load_x(1)            # for qkv(1), emitted in iter 0

            def emit_pending_attnv(pend):
                # attention @ v wave for (g', s') = pend, one sub after its
                # scores so the softmax chain hides under a full sub of
                # filler GEMMs
                sp, p2p, v2p, oTp = pend
                po = [
                    psmall.tile([128, 4, VV], F32, tag=f"att{b4}",
                                name=f"po{b4}",
                                padded_shape=[128, 4, 128])
                    for b4 in range(4)
                ]
                emit_attnv(sp, v2p, p2p, po)
                # evacuate po -> oT (fp16), split DVE/ACT
                oTr = oTp[:].rearrange("p m (b i) -> p m b i", i=VV)
                for b4 in range(4):
                    dst = oTr[:, :, sp * 4 + b4, :]
                    if b4 % 2 == 0:
                        nc.vector.tensor_copy(dst, po[b4][:])
                    else:
                        nc.scalar.activation(
                            dst, po[b4][:],
                            mybir.ActivationFunctionType.Copy,
                        )

            def emit_proj(oTp, nn, t0):
                # pf rides the po banks: its WAR (oT evac of sub3 freeing the
                # bank) coincides with proj's real data dependency on oT
                for co in range(4):
                    pf = psmall.tile([128, NB], F32, tag=f"att{co % 2 * 2 + 1}",
                                     name="pf", padded_shape=[128, 512])
                    for kc in range(4):
                        nc.tensor.matmul(
                            pf[:],
                            wp_r[kc][:, co * 128:(co + 1) * 128],
                            oTp[:, kc, :],
                            start=(kc == 0), stop=(kc == 3),
                        )
                    fin = fpool.tile([128, NB], F16, tag=f"fin{co}", name="fin")
                    if co < 2:
                        nc.vector.tensor_copy(fin[:], pf[:])
                    else:
                        nc.scalar.activation(
                            fin[:], pf[:], mybir.ActivationFunctionType.Copy
                        )
                    nc.sync.dma_start(
                        out=Y[nn, co * 128:(co + 1) * 128, t0:t0 + TG, :],
                        in_=fin[:].rearrange("p (t v) -> p t v", t=TG),
                    )

            # ---- main pipelined loop.  Two pipeline levels: group g+1's
            # qkv GEMMs interleave into group g's attention core as PE
            # filler, and attnv runs one sub behind its scores (attnv(g,3)
            # and proj(g) land in group g+1's sub 0) so every softmax chain
            # (exp->reduce->recip->mul, ~2.5-3us incl. sem-post tails) is
            # covered by ~3us of filler GEMMs with no group-junction stall.
            pending = None          # (s, p2, v2s, oT) awaiting attnv
            proj_todo = None        # (oT, nn, t0) awaiting proj emission
            for g in range(NGROUPS):
                nn = g // (T // TG)
                t0 = (g % (T // TG)) * TG
                have_next = g + 1 < NGROUPS

                if g + 2 < NGROUPS:
                    xp_b = load_x(g + 2)
                else:
                    xp_b = None
                qc_next = [None] * 8
                v2_next = [None] * NSUB
                oT = opool.tile([128, 4, NB], F16, tag="oT", name="oT")

                for s in range(NSUB):
                    # scores wave for (g, s).  PSUM budget: pbig 2 + psm 2 +
                    # po/pf 4 = 8 banks.  psm banks are dedicated so scores
                    # never wait on oT evacuation.
                    psm = [
                        psmall.tile([128, 4, VV], F32, tag=f"psm{par}",
                                    name=f"psm{par}",
                                    padded_shape=[128, 4, 128])
                        for par in range(2)
                    ]
                    emit_scores(s, qc_cur, psm)

                    # softmax over heads (axis split across psm[0]/psm[1])
                    e_t = smpool.tile([128, 2, 4, VV], F16, tag=f"e{s}", name="e_t")
                    for par in range(2):
                        nc.scalar.activation(
                            e_t[:, par], psm[par][:],
                            mybir.ActivationFunctionType.Exp,
                        )

                    # PE filler: next group's qk chunks, front-loaded
                    # [3,2,2,1] across subs so the last chunk's evacuation
                    # (which gates the next group's scores wave) lands a full
                    # sub earlier and ahead of the softmax ops in the DVE
                    # queue
                    sched = ((0, 1, 2), (3, 4), (5, 6), (7,))[s]
                    if have_next:
                        emit_qk_chunk(sched[0], xp_a, qc_next)
                    else:
                        emit_dummy_qk(xp_cur)

                    # previous sub's attention @ v (pipelined one sub deep)
                    if pending is not None:
                        was_tail = pending[0] == NSUB - 1
                        emit_pending_attnv(pending)
                        if was_tail:
                            assert proj_todo is None
                            proj_todo = (pending[3], nn_prev, t0_prev)
                        pending = None

                    # v(g+1, s) here (not after the softmax ops) so its PSUM
                    # slot-WAR resolves a full chunk earlier
                    if have_next:
                        emit_v_sub(s, xp_a, v2_next)
                    else:
                        emit_dummy_v(xp_cur)

                    D = smpool.tile([128, VV], F32, tag=f"D{s}", name="D")
                    nc.vector.reduce_sum(
                        out=D[:],
                        in_=e_t[:].rearrange("p a m i -> p i (a m)"),
                        axis=mybir.AxisListType.X,
                    )
                    rD = smpool.tile([128, VV], F16, tag=f"rD{s}", name="rD")
                    with nc.allow_low_precision(reason="1/D in fp16: D in [2e-2, 3e3], rel err ~5e-4 vs 2e-2 budget"):
                        nc.vector.reciprocal(rD[:], D[:])
                    p2 = smpool.tile([128, 2, 4, VV], F16, tag=f"p2_{s}",
                                     name="p2")
                    nc.vector.tensor_mul(
                        p2[:],
                        e_t[:],
                        rD[:].unsqueeze(1).unsqueeze(1)
                        .broadcast_to([128, 2, 4, VV]),
                    )

                    # PE fillers: next group's remaining qk chunks
                    if have_next:
                        for m in sched[1:]:
                            emit_qk_chunk(m, xp_a, qc_next)
                    else:
                        emit_dummy_qk(xp_cur)

                    # previous group's proj, placed after this sub's fillers
                    # so it never waits on the just-issued oT evacuations
                    if proj_todo is not None:
                        emit_proj(*proj_todo)
                        proj_todo = None

                    pending = (s, p2, v2_cur[s], oT)

                nn_prev, t0_prev = nn, t0
                qc_cur = qc_next
                v2_cur = v2_next
                if xp_a is not None:
                    xp_cur = xp_a
                xp_a = xp_b

            # ---- flush: last group's final attnv + proj ----
            emit_dummy_qk(xp_cur)
            emit_pending_attnv(pending)
            emit_dummy_v(xp_cur)
            emit_proj(pending[3], nn_prev, t0_prev)
    return nc


LAST_RESULT = {}


def kernel(x: np.ndarray, w_qkv: np.ndarray, w_proj: np.ndarray,
           _trace: bool = False) -> np.ndarray:
    n, c, t, vv = x.shape
    assert (n, c, t, vv) == (16, 512, 256, 25)
    scale = np.float32((c // H) ** -0.5)

    wq = w_qkv[:c] * scale
    wk = w_qkv[c:2 * c]
    wv = w_qkv[2 * c:]
    wqkT = np.ascontiguousarray(np.concatenate([wq, wk], axis=0).T.astype(np.float16))
    wvT = np.ascontiguousarray(wv.T.astype(np.float16))
    wprojT = np.ascontiguousarray(w_proj.T.astype(np.float16))

    nc = build_nc()
    split_excess_waits(nc)
    in_maps = []
    for core in range(N_CORES):
        shard = x[core * NN_PER_CORE:(core + 1) * NN_PER_CORE].astype(np.float16)
        xs = np.zeros((NN_PER_CORE, C, T // TG, TG, 32), dtype=np.float16)
        xs[..., :VV] = shard.reshape(NN_PER_CORE, C, T // TG, TG, VV)
        in_maps.append({"x": xs, "wqkT": wqkT, "wvT": wvT, "wprojT": wprojT})

    kw = {}
    if _trace:
        import tempfile
        kw = dict(trace=True, tmpdir=tempfile.mkdtemp(prefix="attn2_trace_"))
    res = run_bass_kernel_spmd(nc, in_maps, list(range(N_CORES)), **kw)
    LAST_RESULT["res"] = res
    LAST_RESULT["tmpdir"] = kw.get("tmpdir")
    out = np.empty((n, c, t, vv), dtype=np.float32)
    for core in range(N_CORES):
        out[core * NN_PER_CORE:(core + 1) * NN_PER_CORE] = \
            res.results[core]["y"].astype(np.float32)
    return out
